# revision 14
# baseline (speedup 1.0000x reference)
"""Trainium2 Bass kernel for nn_K_ANP_41188736369107.

Math: the reference computes
    std = std(x, axis=-1, ddof=1); p = 2 + log1p(mean(std))
    norm = (sum |x|^p)^(1/p); lc = norm/(norm+eps); e = exp(lc)
    out = mean(x*e, -1) / mean(broadcast(e), -1)
Since e is constant along the reduced axis, up/down == mean(x, axis=-1)
exactly (the std/p/norm/exp factors cancel); verified numerically at
~2.6e-7 norm relative error in fp32.  So the kernel is a row-mean over
the last axis (K=64), data-parallel over the batch axis across 8 cores.

Per-core: x[i] (256,512,64) flattened; tile g covers 128 partition
lines of `line` consecutive f32 (line/64 rows of K=64 per partition).
Raw-Bass pipeline (not Tile: Tile embeds two sync-waits into
slot-reusing DMAs, which walrus rejects on DMA_DIRECT2D descriptors):

  SP  seq: in-DMA triggers for even tiles
  ACT seq: in-DMA triggers for odd tiles (second HWDGE queue, hides
           per-trigger latency), scale-by-1/K muls, out-DMAs
  DVE    : row-sum reduce per tile

Schedule: small head tiles (short pipeline fill), 1 MiB steady tiles
(measured stream rate ~2.44-2.53 us/MiB; DVE reduce 2.29 us/MiB rides
just under), small tail tiles (short final reduce + receipt).  Input
ring of 16 slots; slot-free = DVE consumed the previous occupant
(standalone wait_ge, so DMACopy carries zero embedded waits).
Per-slot DMA-completion semaphores: one DMA's 16 increments come from
16 independent SDMA engines, so a shared counter mid-stream would be
racy; the single final out_sem wait targets the full total, which is
race-free.  Sum/out tiles are dedicated per tile (outputs are tiny),
so DVE never waits on downstream stages.
"""

from contextlib import ExitStack

import numpy as np

import concourse.bass as bass
import concourse.mybir as mybir
from concourse.bass_utils import run_bass_kernel_spmd

K = 64          # reduced (neighbor) axis
P = 128         # SBUF partitions
N_CORES = 8
B, C, G = 8, 256, 512   # knn_x_w shape is (B, C, G, K)

ELEMS = C * G * K            # 8,388,608 f32 per core
LINES = ELEMS // P           # 65,536 f32 per partition

HEAD = 1024                  # head small-tile line length (512 KiB tile)
TAIL = 512                   # tail small-tile line length (256 KiB tile)
BIG = 2048                   # steady-tile line length (1 MiB tile)
N_HEAD = 4
N_TAIL = 8
N_BIG = (LINES - N_HEAD * HEAD - N_TAIL * TAIL) // BIG   # 28
SCHEDULE = [HEAD] * N_HEAD + [BIG] * N_BIG + [TAIL] * N_TAIL
assert sum(SCHEDULE) == LINES

NBIGBUF = 16    # big-tile ring slots (16 * 8 KiB/partition)

F32 = mybir.dt.float32


def build_nc(schedule=None, nbigbuf=NBIGBUF):
    schedule = list(SCHEDULE if schedule is None else schedule)
    n = len(schedule)

    # Bass.__init__ emits four const memsets plus an all-engine barrier
    # (~3.5us EVSEM butterfly + drain) before any user code.  Nothing in
    # this kernel reads those consts, so skip the init barrier; the
    # Block-exit barrier (needed for completion) is emitted after the
    # patch is restored.
    _orig_barrier = bass.Bass.all_engine_barrier
    bass.Bass.all_engine_barrier = lambda self, *a, **k: None
    try:
        nc = bass.Bass()
    finally:
        bass.Bass.all_engine_barrier = _orig_barrier

    lines = sum(schedule)
    x = nc.dram_tensor("x", [P * lines], F32, kind="ExternalInput")
    y = nc.dram_tensor("y", [2 * lines], F32, kind="ExternalOutput")

    with ExitStack() as ctx:
        # per-tile static metadata; big tiles share a ring of nbigbuf slots,
        # small head/tail tiles get dedicated buffers (they are small)
        tiles = []
        prefix = 0
        bi = 0       # big-tile ordinal
        smi = 0      # small-tile ordinal
        big_bufs = [
            ctx.enter_context(nc.sbuf_tensor(f"bgb{i}", [P, BIG], F32))
            for i in range(nbigbuf)
        ]
        bg_sems = [
            ctx.enter_context(nc.semaphore(f"bg_sem{i}")) for i in range(nbigbuf)
        ]
        for g, line in enumerate(schedule):
            rows = line // K
            if line == BIG:
                s = bi % nbigbuf
                buf, sem = big_bufs[s], bg_sems[s]
                uses = bi // nbigbuf + 1
                slot_prev = g - nbigbuf if bi >= nbigbuf else None
                bi += 1
            else:
                buf = ctx.enter_context(
                    nc.sbuf_tensor(f"smb{smi}", [P, line], F32)
                )
                sem = ctx.enter_context(nc.semaphore(f"sm_sem{smi}"))
                uses, slot_prev = 1, None
                smi += 1
            st = ctx.enter_context(nc.sbuf_tensor(f"st{g}", [P, rows], F32))
            yt = ctx.enter_context(nc.sbuf_tensor(f"yt{g}", [P, rows], F32))
            tiles.append(
                dict(
                    g=g, line=line, rows=rows, prefix=prefix, buf=buf,
                    sem=sem, uses=uses, slot_prev=slot_prev, st=st, yt=yt,
                )
            )
            prefix += line

        dve_sem = ctx.enter_context(nc.semaphore("dve_sem"))
        mul_sem = ctx.enter_context(nc.semaphore("mul_sem"))
        out_sem = ctx.enter_context(nc.semaphore("out_sem"))

        def in_src(tl):
            return x[
                P * tl["prefix"] : P * (tl["prefix"] + tl["line"])
            ].rearrange("(p l) -> p l", l=tl["line"])

        def trigger_in(eng, tl):
            eng.dma_start(tl["buf"][:], in_src(tl)).then_inc(tl["sem"], 16)

        sp_tiles = [tl for tl in tiles if tl["g"] % 2 == 0]
        act_tiles = [tl for tl in tiles if tl["g"] % 2 == 1]

        with nc.Block(no_gpsimd_drain=True) as block:

            @block.sync
            def _(sp):
                for tl in sp_tiles:
                    if tl["slot_prev"] is not None:
                        # slot free once DVE consumed its previous occupant
                        # (transitively implies that tile's DMA completed)
                        sp.wait_ge(dve_sem, tl["slot_prev"] + 1)
                    trigger_in(sp, tl)

            @block.vector
            def _(v):
                for tl in tiles:
                    v.wait_ge(tl["sem"], 16 * tl["uses"])
                    view = tl["buf"][:].rearrange("p (r k) -> p r k", k=K)
                    v.reduce_sum(
                        tl["st"][:], view, axis=mybir.AxisListType.X
                    ).then_inc(dve_sem, 1)

            @block.scalar
            def _(act):
                # prologue: odd tiles whose ring slot is fresh fire up front
                pending = [tl for tl in act_tiles]
                for tl in list(pending):
                    if tl["slot_prev"] is None and tl["g"] < 2 * nbigbuf:
                        trigger_in(act, tl)
                        pending.remove(tl)
                by_iter = {}
                for tl in pending:
                    # reused-slot tiles go at iteration slot_prev, whose
                    # dve_sem wait IS their slot-free condition; static-
                    # buffer tails keep the same 16-tile lookahead
                    it = (
                        tl["slot_prev"]
                        if tl["slot_prev"] is not None
                        else tl["g"] - nbigbuf
                    )
                    by_iter.setdefault(it, []).append(tl)
                for g, tl in enumerate(tiles):
                    act.wait_ge(dve_sem, g + 1)
                    for h in by_iter.get(g, ()):
                        trigger_in(act, h)
                    act.mul(tl["yt"][:], tl["st"][:], 1.0 / K).then_inc(mul_sem, 1)
                    # seq-side wait for our own mul before triggering the DMA
                    act.wait_ge(mul_sem, g + 1)
                    dst = y[
                        2 * tl["prefix"] : 2 * tl["prefix"] + P * tl["rows"]
                    ].rearrange("(p r) -> p r", r=tl["rows"])
                    act.dma_start(dst, tl["yt"][:]).then_inc(out_sem, 16)
                # all out-DMAs ride this queue; per-SDMA-engine FIFO means the
                # final total implies every earlier output landed
                act.wait_ge(out_sem, 16 * n)

    return nc


_NC_CACHE = {}


def _get_nc():
    if "nc" not in _NC_CACHE:
        _NC_CACHE["nc"] = build_nc()
    return _NC_CACHE["nc"]


def _run(x, trace=False, tmpdir=None):
    """x: (B, C, G, K) float32 -> (B, C, G) float32.  Returns (out, results)."""
    x = np.ascontiguousarray(x, dtype=np.float32)
    assert x.shape == (B, C, G, K), x.shape
    nc = _get_nc()
    in_maps = [{"x": x[i].reshape(-1)} for i in range(N_CORES)]
    res = run_bass_kernel_spmd(
        nc, in_maps, core_ids=list(range(N_CORES)), trace=trace, tmpdir=tmpdir
    )
    out = np.stack(
        [res.results[i]["y"].reshape(C, G) for i in range(N_CORES)], axis=0
    )
    return out, res


def kernel(**inputs):
    out, _ = _run(inputs["knn_x_w"])
    return out
